# revision 20
# baseline (speedup 1.0000x reference)
"""YOLO-style detection head decode on 8 Trainium2 NeuronCores.

Input : x [64, 255, 52, 52] f32
Output: [64, 8112, 85] f32  (bbox(4) | conf(1) | cls(80), sigmoid/exp decoded)

The kernel is DMA-bound (360 B/ns shared DMA engines), so everything rides
the wire as fp16 (rel err ~2e-3 vs the 2e-2 gate; bf16's 7-bit mantissa
fails at sigmoid tails).  Per core: 8 batches x 3 anchors = 24 slabs.

  - host packs xe [8, 3, 85, 2706] fp16 per slab (grid padded 2704->2706):
      rows 0/1  = raw tw, th          (decoded by the staged exp below)
      rows 2/3  = raw tx, ty          (sigmoid applied POST-transpose)
      rows 4..84 = raw conf, cls0..79 (sigmoid applied POST-transpose)
  - tw/th exp(+ln anchor bias) runs as ONE [48, 2706] ACT op on a staging
    tile (a per-slab [2, 2706] op would cost the same 2.3us each on ACT's
    free-dim clock), then moves into slab rows 0:2 by SBUF->SBUF DMA on the
    otherwise-idle Pool engine (24 transfers; keep this count low — each
    SWDGE op costs ~1us of Pool descriptor-generation time and the fill
    phase advances at that pace).
  - per slab: one 83-row HBM load (rows 2:85), 22 fp16 matmuls (+2 dummies
    to fill the 24-group psum) transpose to output layout: lhsT free dim
    strided by 22 so each of 123 output partitions holds 22 consecutive
    output rows; rhs = constant [85, 85] permutation (exp rows -> cols 2/3,
    tx/ty -> cols 0/1, raw rows -> cols 4..84).
  - psum [123, 4, 512] f32 (4 banks, 6 x 85-col groups per bank) drains via
    ONE whole-tile ACT sigmoid [123, (4, 510)] f32->fp16 — decoding conf,
    cls AND tx/ty while transposing — then per bank a small DVE copy
    re-overwrites exp cols 2:4 of each group with raw psum values, and two
    DVE ops finish bbox xy: out[:, t, 0:2] = 8*sigmoid + 8*grid (pattern
    add from a tiny constant tile).
  - out DRAM padded to [.., 2706, 85] so each slab stores with a single
    uniform [123 x 3740B] DMA issued from SP two slabs late (so its
    semaphore wait never blocks the next loads' issue); host drops the 2
    pad rows when gathering.
"""

import numpy as np

G = 52
GG = G * G  # 2704
A = 3
NCH = 85  # 5 + 80
B = 64
N_CORES = 8
B_PER_CORE = B // N_CORES  # 8
STRIDE = 8.0  # 416 / 52
ANCHORS_PX = np.array([[10.0, 13.0], [16.0, 30.0], [33.0, 23.0]], dtype=np.float32)
K_MM = 85  # 2 exp + 2 raw xy + 81 raw conf/cls
R = 22  # output rows per partition
P_OUT = 123  # output partitions per matmul (123*22 = 2706 >= 2704)
FREE = P_OUT * R  # 2706
N_SLABS = B_PER_CORE * A  # 24
NGRP = 24  # psum groups per slab (22 real + 2 dummy)
OUT_COLS = NGRP * NCH  # 2040

_CACHE = {}


def _build_consts():
    mmat = np.zeros((K_MM, NCH), dtype=np.float16)
    mmat[0, 2] = 1.0  # exp(tw)*aw_px -> col 2
    mmat[1, 3] = 1.0  # exp(th)*ah_px -> col 3
    mmat[2, 0] = 1.0  # raw tx -> col 0 (sigmoid'd at drain)
    mmat[3, 1] = 1.0  # raw ty -> col 1
    for k in range(81):  # raw conf + cls -> cols 4..84
        mmat[4 + k, 4 + k] = 1.0

    # 8*cx / 8*cy for grid row 22p + t (multiples of 8: exact in fp16)
    cxpat = np.zeros((P_OUT, NGRP, 2), dtype=np.float16)
    p = np.arange(P_OUT)[:, None]
    t = np.arange(R)[None, :]
    row = R * p + t  # [123, 22]
    valid = row < GG
    cxpat[:, :R, 0] = np.where(valid, STRIDE * (row % G), 0).astype(np.float16)
    cxpat[:, :R, 1] = np.where(valid, STRIDE * (row // G), 0).astype(np.float16)
    return mmat, cxpat


def build_nc():
    if "nc" in _CACHE:
        return _CACHE["nc"]
    from contextlib import ExitStack

    import concourse.bacc as bacc
    import concourse.tile as tile
    from concourse import mybir
    from concourse.tile_rust import add_dep_helper

    AF = mybir.ActivationFunctionType
    dt = mybir.dt

    nc = bacc.Bacc("TRN2", target_bir_lowering=False, debug=False)
    xe_t = nc.dram_tensor(
        "xe", [B_PER_CORE, A, K_MM, FREE], dt.float16, kind="ExternalInput"
    )
    mmat_t = nc.dram_tensor("mmat", [K_MM, NCH], dt.float16, kind="ExternalInput")
    cxpat_t = nc.dram_tensor(
        "cxpat", [P_OUT, NGRP * 2], dt.float16, kind="ExternalInput"
    )
    out_t = nc.dram_tensor(
        "out", [B_PER_CORE, A, FREE, NCH], dt.float16, kind="ExternalOutput"
    )
    xe_ap = xe_t.ap()
    mmat_ap = mmat_t.ap()
    cxpat_ap = cxpat_t.ap()
    out_ap = out_t.ap()

    with ExitStack() as ctx:
        tc = ctx.enter_context(tile.TileContext(nc))
        singles = ctx.enter_context(tc.tile_pool(name="singles", bufs=1))
        slabs = ctx.enter_context(tc.tile_pool(name="slabs", bufs=14))
        outs = ctx.enter_context(tc.tile_pool(name="outs", bufs=5))
        psums = ctx.enter_context(tc.tile_pool(name="psum", bufs=2, space="PSUM"))

        stg_exp = singles.tile([2 * N_SLABS, FREE], dt.float16)
        mmat_sb = singles.tile([K_MM, NCH], dt.float16)
        cxpat_sb = singles.tile([P_OUT, NGRP * 2], dt.float16)

        # SP issues the exp staging input first (it gates the Pool s2s chain
        # and so the whole pipeline fill); everything stays off ACT's
        # sequencer so the exp table load is ACT's first instruction at t~0
        # anchor scale rides in the host pack (tw' = tw + ln(anchor_px)), so
        # the staged exp needs no bias and starts as soon as its rows land
        nc.sync.dma_start(out=stg_exp[:, :], in_=xe_ap[:, :, 0:2, :])
        # const issues ride ACT's sequencer (hidden inside exp's wait for its
        # staging rows) so SP goes straight from staging to the slab loads
        nc.scalar.dma_start(out=mmat_sb[:, :], in_=mmat_ap[:, :])
        nc.scalar.dma_start(out=cxpat_sb[:, :], in_=cxpat_ap[:, :])
        nc.scalar.activation(stg_exp[:, :], stg_exp[:, :], AF.Exp)

        # warm the PE (pipeline + p-state) on the constant matrix while the
        # first slab loads stream in
        wps = psums.tile([P_OUT, 4, 512], dt.float32, tag="ps")
        for _ in range(16):
            nc.tensor.matmul(
                wps[0:NCH, 0, 0:NCH], mmat_sb[:, :], mmat_sb[:, :],
                start=True, stop=True,
            )

        # stores issue from SP two slabs late so their semaphore waits never
        # block the next loads' issue (HWDGE here is SP/ACT only; ACT's
        # sequencer has no slack and Pool's SWDGE engine time is too dear)
        pending_stores = []
        s2s_hist = []

        def flush_store():
            bb, aa, hh, sb_tile = pending_stores.pop(0)
            nc.sync.dma_start(
                out=out_ap[bb, aa, :, :].rearrange("(p r) c -> p (r c)", r=R),
                in_=sb_tile[:, 0 : R * NCH],
            )

        for b in range(B_PER_CORE):
            for a in range(A):
                s = A * b + a
                slab = slabs.tile([K_MM, FREE], dt.float16)
                # exp rows move by DMA on the Pool engine (engine copies need
                # 32-aligned partition bases)
                s2s_i = nc.gpsimd.dma_start(
                    out=slab[0:2, :], in_=stg_exp[2 * s : 2 * s + 2, :]
                )
                s2s_hist.append(s2s_i)
                load_i = nc.sync.dma_start(
                    out=slab[2:K_MM, :], in_=xe_ap[b, a, 2:K_MM, :]
                )
                if s >= 3:
                    # the DMA engines are an exclusive FIFO: without this, the
                    # tiny s2s transfers queue behind every already-issued
                    # slab load during pipeline fill and the first matmuls
                    # (and everything behind them) start ~5us late
                    add_dep_helper(
                        load_i.ins, s2s_hist[s - 3].ins, sync=True,
                        reason="fill: exp-row transfers jump the load queue",
                    )
                if len(pending_stores) >= 5:
                    flush_store()
                # [K_MM, P_OUT, R]: free index (p, t) -> grid row R*p + t
                slab_r = slab[:, :].rearrange("k (p t) -> k p t", t=R)

                ps = psums.tile([P_OUT, 4, 512], dt.float32, tag="ps")
                for k in range(NGRP):
                    # full 123 partitions even for t>=20: pad cols of xe are
                    # zero, so the 2 out-of-range grid rows compute to benign
                    # values (excluded from the store DMA); groups 22/23 are
                    # dummies that keep the psum tile uniformly initialized
                    # for the whole-tile sigmoid drain below
                    t = k if k < R else 0
                    bank, jj = divmod(k, 6)
                    nc.tensor.matmul(
                        ps[:, bank, jj * NCH : (jj + 1) * NCH],
                        slab_r[:, :, t],
                        mmat_sb[:, :],
                        start=True,
                        stop=True,
                    )

                out_sb = outs.tile([P_OUT, OUT_COLS], dt.float16)
                # drains split per bank-pair so the first half's sigmoid
                # overlaps the second half's matmuls; each half: one ACT
                # sigmoid decodes conf/cls AND tx/ty (cols 0:2, 4:85 of every
                # group) while converting f32->fp16, exp cols get sigmoid'd
                # garbage and are re-overwritten from psum by the DVE copy,
                # then two DVE ops finish bbox xy: 8*sigmoid + 8*grid offset
                for h in range(2):
                    cols = slice(h * 12 * NCH, (h + 1) * 12 * NCH)
                    nc.scalar.activation(
                        out_sb[:, cols].rearrange("p (b c) -> p b c", c=6 * NCH),
                        ps[:, 2 * h : 2 * h + 2, 0 : 6 * NCH],
                        AF.Sigmoid,
                    )
                    nc.vector.tensor_copy(
                        out_sb[:, cols].rearrange(
                            "p (b g c) -> p b g c", b=2, c=NCH
                        )[:, :, :, 2:4],
                        ps[:, 2 * h : 2 * h + 2, 0 : 6 * NCH].rearrange(
                            "p b (g c) -> p b g c", c=NCH
                        )[:, :, :, 2:4],
                    )
                    xy = out_sb[:, cols].rearrange("p (g c) -> p g c", c=NCH)[
                        :, :, 0:2
                    ]
                    nc.vector.tensor_scalar_mul(xy, xy, 8.0)
                    nc.vector.tensor_tensor(
                        xy,
                        xy,
                        cxpat_sb[:, 24 * h : 24 * (h + 1)].rearrange(
                            "p (g c) -> p g c", c=2
                        ),
                        mybir.AluOpType.add,
                    )
                # uniform [123 x 3740B] store; DRAM rows 2704:2706 are pad
                pending_stores.append((b, a, 0, out_sb))
        while pending_stores:
            flush_store()

    nc.compile()
    _CACHE["nc"] = nc
    return nc


def _pack_core_input(x_core):
    """x_core [B_PER_CORE, 255, 52, 52] f32 -> xe [B_PER_CORE, A, 85, FREE] f16."""
    xr = x_core.reshape(B_PER_CORE, A, NCH, GG)
    xe = np.zeros((B_PER_CORE, A, K_MM, FREE), dtype=np.float16)
    law = np.log(ANCHORS_PX[:, 0:1]).astype(np.float32)[None, :, :, None]
    lah = np.log(ANCHORS_PX[:, 1:2]).astype(np.float32)[None, :, :, None]
    xe[:, :, 0:1, 0:GG] = xr[:, :, 2:3] + law  # tw + ln(anchor_w_px)
    xe[:, :, 1:2, 0:GG] = xr[:, :, 3:4] + lah  # th + ln(anchor_h_px)
    xe[:, :, 2:4, 0:GG] = xr[:, :, 0:2]  # tx, ty
    xe[:, :, 4:NCH, 0:GG] = xr[:, :, 4:NCH]  # conf, cls
    return xe


def kernel(x):
    x = np.ascontiguousarray(np.asarray(x), dtype=np.float32)
    assert x.shape == (B, A * NCH, G, G), x.shape
    nc = build_nc()
    from concourse.bass_utils import run_bass_kernel_spmd

    mmat, cxpat = _build_consts()
    in_maps = []
    for c in range(N_CORES):
        in_maps.append(
            {
                "xe": _pack_core_input(x[c * B_PER_CORE : (c + 1) * B_PER_CORE]),
                "mmat": mmat,
                "cxpat": cxpat.reshape(P_OUT, NGRP * 2),
            }
        )
    # transient NRT_EXEC_UNIT_UNRECOVERABLE has been observed once on a cold
    # first execution and never again; retry a couple of times before failing
    for attempt in range(3):
        try:
            res = run_bass_kernel_spmd(nc, in_maps, core_ids=list(range(N_CORES)))
            break
        except Exception:  # noqa: BLE001
            if attempt == 2:
                raise
            import time

            time.sleep(2.0 * (attempt + 1))
    _CACHE["last_res"] = res
    out = np.concatenate([r["out"] for r in res.results], axis=0)
    out = out[:, :, 0:GG, :].astype(np.float32)
    return out.reshape(B, A * GG, NCH)


# revision 21
# speedup vs baseline: 1.0824x; 1.0824x over previous
"""YOLO-style detection head decode on 8 Trainium2 NeuronCores.

Input : x [64, 255, 52, 52] f32
Output: [64, 8112, 85] f32  (bbox(4) | conf(1) | cls(80), sigmoid/exp decoded)

The kernel is DMA-bound (360 B/ns shared DMA engines), so everything rides
the wire as fp16 (rel err ~2e-3 vs the 2e-2 gate; bf16's 7-bit mantissa
fails at sigmoid tails).  Per core: 8 batches x 3 anchors = 24 slabs.

  - host packs xe [8, 3, 85, 2706] fp16 per slab (grid padded 2704->2706):
      rows 0/1  = raw tw, th          (decoded by the staged exp below)
      rows 2/3  = raw tx, ty          (sigmoid applied POST-transpose)
      rows 4..84 = raw conf, cls0..79 (sigmoid applied POST-transpose)
  - tw/th exp(+ln anchor bias) runs as ONE [48, 2706] ACT op on a staging
    tile (a per-slab [2, 2706] op would cost the same 2.3us each on ACT's
    free-dim clock), then moves into slab rows 0:2 by SBUF->SBUF DMA on the
    otherwise-idle Pool engine (24 transfers; keep this count low — each
    SWDGE op costs ~1us of Pool descriptor-generation time and the fill
    phase advances at that pace).
  - per slab: one 83-row HBM load (rows 2:85), 22 fp16 matmuls (+2 dummies
    to fill the 24-group psum) transpose to output layout: lhsT free dim
    strided by 22 so each of 123 output partitions holds 22 consecutive
    output rows; rhs = constant [85, 85] permutation (exp rows -> cols 2/3,
    tx/ty -> cols 0/1, raw rows -> cols 4..84).
  - psum [123, 4, 512] f32 (4 banks, 6 x 85-col groups per bank) drains via
    ONE whole-tile ACT sigmoid [123, (4, 510)] f32->fp16 — decoding conf,
    cls AND tx/ty while transposing — then per bank a small DVE copy
    re-overwrites exp cols 2:4 of each group with raw psum values, and two
    DVE ops finish bbox xy: out[:, t, 0:2] = 8*sigmoid + 8*grid (pattern
    add from a tiny constant tile).
  - out DRAM padded to [.., 2706, 85] so each slab stores with a single
    uniform [123 x 3740B] DMA issued from SP two slabs late (so its
    semaphore wait never blocks the next loads' issue); host drops the 2
    pad rows when gathering.
"""

import numpy as np

G = 52
GG = G * G  # 2704
A = 3
NCH = 85  # 5 + 80
B = 64
N_CORES = 8
B_PER_CORE = B // N_CORES  # 8
STRIDE = 8.0  # 416 / 52
ANCHORS_PX = np.array([[10.0, 13.0], [16.0, 30.0], [33.0, 23.0]], dtype=np.float32)
K_MM = 85  # 2 exp + 2 raw xy + 81 raw conf/cls
R = 22  # output rows per partition
P_OUT = 123  # output partitions per matmul (123*22 = 2706 >= 2704)
FREE = P_OUT * R  # 2706
N_SLABS = B_PER_CORE * A  # 24
NGRP = 24  # psum groups per slab (22 real + 2 dummy)
OUT_COLS = NGRP * NCH  # 2040

_CACHE = {}


def _build_consts():
    mmat = np.zeros((K_MM, NCH), dtype=np.float16)
    mmat[0, 2] = 1.0  # exp(tw)*aw_px -> col 2
    mmat[1, 3] = 1.0  # exp(th)*ah_px -> col 3
    mmat[2, 0] = 1.0  # raw tx -> col 0 (sigmoid'd at drain)
    mmat[3, 1] = 1.0  # raw ty -> col 1
    for k in range(81):  # raw conf + cls -> cols 4..84
        mmat[4 + k, 4 + k] = 1.0

    # 8*cx / 8*cy for grid row 22p + t (multiples of 8: exact in fp16)
    cxpat = np.zeros((P_OUT, NGRP, 2), dtype=np.float16)
    p = np.arange(P_OUT)[:, None]
    t = np.arange(R)[None, :]
    row = R * p + t  # [123, 22]
    valid = row < GG
    cxpat[:, :R, 0] = np.where(valid, STRIDE * (row % G), 0).astype(np.float16)
    cxpat[:, :R, 1] = np.where(valid, STRIDE * (row // G), 0).astype(np.float16)
    return mmat, cxpat


def build_nc():
    if "nc" in _CACHE:
        return _CACHE["nc"]
    from contextlib import ExitStack

    import concourse.bacc as bacc
    import concourse.tile as tile
    from concourse import mybir
    from concourse.tile_rust import add_dep_helper

    AF = mybir.ActivationFunctionType
    dt = mybir.dt

    nc = bacc.Bacc("TRN2", target_bir_lowering=False, debug=False)
    xe_t = nc.dram_tensor(
        "xe", [B_PER_CORE, A, K_MM, FREE], dt.float16, kind="ExternalInput"
    )
    mmat_t = nc.dram_tensor("mmat", [K_MM, NCH], dt.float16, kind="ExternalInput")
    cxpat_t = nc.dram_tensor(
        "cxpat", [P_OUT, NGRP * 2], dt.float16, kind="ExternalInput"
    )
    out_t = nc.dram_tensor(
        "out", [B_PER_CORE, A, FREE, NCH], dt.float16, kind="ExternalOutput"
    )
    xe_ap = xe_t.ap()
    mmat_ap = mmat_t.ap()
    cxpat_ap = cxpat_t.ap()
    out_ap = out_t.ap()

    with ExitStack() as ctx:
        tc = ctx.enter_context(tile.TileContext(nc))
        singles = ctx.enter_context(tc.tile_pool(name="singles", bufs=1))
        slabs = ctx.enter_context(tc.tile_pool(name="slabs", bufs=14))
        outs = ctx.enter_context(tc.tile_pool(name="outs", bufs=5))
        psums = ctx.enter_context(tc.tile_pool(name="psum", bufs=2, space="PSUM"))

        stg_exp = singles.tile([2 * N_SLABS, FREE], dt.float16)
        mmat_sb = singles.tile([K_MM, NCH], dt.float16)
        cxpat_sb = singles.tile([P_OUT, NGRP * 2], dt.float16)

        # SP issues the exp staging input first (it gates the Pool s2s chain
        # and so the whole pipeline fill); everything stays off ACT's
        # sequencer so the exp table load is ACT's first instruction at t~0
        # anchor scale rides in the host pack (tw' = tw + ln(anchor_px)), so
        # the staged exp needs no bias and starts as soon as its rows land
        nc.sync.dma_start(out=stg_exp[:, :], in_=xe_ap[:, :, 0:2, :])
        # const issues ride ACT's sequencer (hidden inside exp's wait for its
        # staging rows) so SP goes straight from staging to the slab loads
        nc.scalar.dma_start(out=mmat_sb[:, :], in_=mmat_ap[:, :])
        nc.scalar.dma_start(out=cxpat_sb[:, :], in_=cxpat_ap[:, :])
        nc.scalar.activation(stg_exp[:, :], stg_exp[:, :], AF.Exp)

        # warm the PE (pipeline + p-state) on the constant matrix while the
        # first slab loads stream in
        wps = psums.tile([P_OUT, 4, 512], dt.float32, tag="ps")
        for _ in range(16):
            nc.tensor.matmul(
                wps[0:NCH, 0, 0:NCH], mmat_sb[:, :], mmat_sb[:, :],
                start=True, stop=True,
            )

        # stores issue from SP two slabs late so their semaphore waits never
        # block the next loads' issue (HWDGE here is SP/ACT only; ACT's
        # sequencer has no slack and Pool's SWDGE engine time is too dear)
        pending_stores = []
        s2s_hist = []

        def flush_store():
            bb, aa, hh, sb_tile = pending_stores.pop(0)
            nc.sync.dma_start(
                out=out_ap[bb, aa, :, :].rearrange("(p r) c -> p (r c)", r=R),
                in_=sb_tile[:, 0 : R * NCH],
            )

        for b in range(B_PER_CORE):
            for a in range(A):
                s = A * b + a
                slab = slabs.tile([K_MM, FREE], dt.float16)
                # exp rows move by DMA on the Pool engine (engine copies need
                # 32-aligned partition bases)
                s2s_i = nc.gpsimd.dma_start(
                    out=slab[0:2, :], in_=stg_exp[2 * s : 2 * s + 2, :]
                )
                s2s_hist.append(s2s_i)
                load_i = nc.sync.dma_start(
                    out=slab[2:K_MM, :], in_=xe_ap[b, a, 2:K_MM, :]
                )
                if s >= 3:
                    # the DMA engines are an exclusive FIFO: without this, the
                    # tiny s2s transfers queue behind every already-issued
                    # slab load during pipeline fill and the first matmuls
                    # (and everything behind them) start ~5us late
                    add_dep_helper(
                        load_i.ins, s2s_hist[s - 3].ins, sync=True,
                        reason="fill: exp-row transfers jump the load queue",
                    )
                if len(pending_stores) >= 4:
                    flush_store()
                # [K_MM, P_OUT, R]: free index (p, t) -> grid row R*p + t
                slab_r = slab[:, :].rearrange("k (p t) -> k p t", t=R)

                ps = psums.tile([P_OUT, 4, 512], dt.float32, tag="ps")
                for k in range(NGRP):
                    # full 123 partitions even for t>=20: pad cols of xe are
                    # zero, so the 2 out-of-range grid rows compute to benign
                    # values (excluded from the store DMA); groups 22/23 are
                    # dummies that keep the psum tile uniformly initialized
                    # for the whole-tile sigmoid drain below
                    t = k if k < R else 0
                    bank, jj = divmod(k, 6)
                    nc.tensor.matmul(
                        ps[:, bank, jj * NCH : (jj + 1) * NCH],
                        slab_r[:, :, t],
                        mmat_sb[:, :],
                        start=True,
                        stop=True,
                    )

                out_sb = outs.tile([P_OUT, OUT_COLS], dt.float16)
                # one whole-tile drain decodes conf/cls AND tx/ty (cols 0:2,
                # 4:85 of every group) while converting f32->fp16; exp cols
                # get sigmoid'd garbage here and are re-overwritten below
                nc.scalar.activation(
                    out_sb[:, :].rearrange("p (b c) -> p b c", c=6 * NCH),
                    ps[:, :, 0 : 6 * NCH],
                    AF.Sigmoid,
                )
                nc.vector.tensor_copy(
                    out_sb[:, :].rearrange("p (b g c) -> p b g c", b=4, c=NCH)[
                        :, :, :, 2:4
                    ],
                    ps[:, :, 0 : 6 * NCH].rearrange("p b (g c) -> p b g c", c=NCH)[
                        :, :, :, 2:4
                    ],
                )
                # bbox xy: 8*sigmoid(t) + 8*grid offset
                xy = out_sb[:, :].rearrange("p (g c) -> p g c", c=NCH)[:, :, 0:2]
                nc.vector.tensor_scalar_mul(xy, xy, 8.0)
                nc.vector.tensor_tensor(
                    xy,
                    xy,
                    cxpat_sb[:, :].rearrange("p (g c) -> p g c", c=2),
                    mybir.AluOpType.add,
                )
                # uniform [123 x 3740B] store; DRAM rows 2704:2706 are pad
                pending_stores.append((b, a, 0, out_sb))
        while pending_stores:
            flush_store()

    nc.compile()
    _CACHE["nc"] = nc
    return nc


def _pack_core_input(x_core):
    """x_core [B_PER_CORE, 255, 52, 52] f32 -> xe [B_PER_CORE, A, 85, FREE] f16."""
    xr = x_core.reshape(B_PER_CORE, A, NCH, GG)
    xe = np.zeros((B_PER_CORE, A, K_MM, FREE), dtype=np.float16)
    law = np.log(ANCHORS_PX[:, 0:1]).astype(np.float32)[None, :, :, None]
    lah = np.log(ANCHORS_PX[:, 1:2]).astype(np.float32)[None, :, :, None]
    xe[:, :, 0:1, 0:GG] = xr[:, :, 2:3] + law  # tw + ln(anchor_w_px)
    xe[:, :, 1:2, 0:GG] = xr[:, :, 3:4] + lah  # th + ln(anchor_h_px)
    xe[:, :, 2:4, 0:GG] = xr[:, :, 0:2]  # tx, ty
    xe[:, :, 4:NCH, 0:GG] = xr[:, :, 4:NCH]  # conf, cls
    return xe


def kernel(x):
    x = np.ascontiguousarray(np.asarray(x), dtype=np.float32)
    assert x.shape == (B, A * NCH, G, G), x.shape
    nc = build_nc()
    from concourse.bass_utils import run_bass_kernel_spmd

    mmat, cxpat = _build_consts()
    in_maps = []
    for c in range(N_CORES):
        in_maps.append(
            {
                "xe": _pack_core_input(x[c * B_PER_CORE : (c + 1) * B_PER_CORE]),
                "mmat": mmat,
                "cxpat": cxpat.reshape(P_OUT, NGRP * 2),
            }
        )
    # transient NRT_EXEC_UNIT_UNRECOVERABLE has been observed once on a cold
    # first execution and never again; retry a couple of times before failing
    for attempt in range(3):
        try:
            res = run_bass_kernel_spmd(nc, in_maps, core_ids=list(range(N_CORES)))
            break
        except Exception:  # noqa: BLE001
            if attempt == 2:
                raise
            import time

            time.sleep(2.0 * (attempt + 1))
    _CACHE["last_res"] = res
    out = np.concatenate([r["out"] for r in res.results], axis=0)
    out = out[:, :, 0:GG, :].astype(np.float32)
    return out.reshape(B, A * GG, NCH)


# revision 22
# speedup vs baseline: 1.0970x; 1.0135x over previous
"""YOLO-style detection head decode on 8 Trainium2 NeuronCores.

Input : x [64, 255, 52, 52] f32
Output: [64, 8112, 85] f32  (bbox(4) | conf(1) | cls(80), sigmoid/exp decoded)

The kernel is DMA-bound (360 B/ns shared DMA engines), so everything rides
the wire as fp16 (rel err ~2e-3 vs the 2e-2 gate; bf16's 7-bit mantissa
fails at sigmoid tails).  Per core: 8 batches x 3 anchors = 24 slabs.

  - host packs xe [8, 3, 85, 2706] fp16 per slab (grid padded 2704->2706):
      rows 0/1  = raw tw, th          (decoded by the staged exp below)
      rows 2/3  = raw tx, ty          (sigmoid applied POST-transpose)
      rows 4..84 = raw conf, cls0..79 (sigmoid applied POST-transpose)
  - tw/th exp(+ln anchor bias) runs as ONE [48, 2706] ACT op on a staging
    tile (a per-slab [2, 2706] op would cost the same 2.3us each on ACT's
    free-dim clock), then moves into slab rows 0:2 by SBUF->SBUF DMA on the
    otherwise-idle Pool engine (24 transfers; keep this count low — each
    SWDGE op costs ~1us of Pool descriptor-generation time and the fill
    phase advances at that pace).
  - per slab: one 83-row HBM load (rows 2:85), 22 fp16 matmuls (+2 dummies
    to fill the 24-group psum) transpose to output layout: lhsT free dim
    strided by 22 so each of 123 output partitions holds 22 consecutive
    output rows; rhs = constant [85, 85] permutation (exp rows -> cols 2/3,
    tx/ty -> cols 0/1, raw rows -> cols 4..84).
  - psum [123, 4, 512] f32 (4 banks, 6 x 85-col groups per bank) drains via
    ONE whole-tile ACT sigmoid [123, (4, 510)] f32->fp16 — decoding conf,
    cls AND tx/ty while transposing — then per bank a small DVE copy
    re-overwrites exp cols 2:4 of each group with raw psum values, and two
    DVE ops finish bbox xy: out[:, t, 0:2] = 8*sigmoid + 8*grid (pattern
    add from a tiny constant tile).
  - out DRAM padded to [.., 2706, 85] so each slab stores with a single
    uniform [123 x 3740B] DMA issued from SP two slabs late (so its
    semaphore wait never blocks the next loads' issue); host drops the 2
    pad rows when gathering.
"""

import numpy as np

G = 52
GG = G * G  # 2704
A = 3
NCH = 85  # 5 + 80
B = 64
N_CORES = 8
B_PER_CORE = B // N_CORES  # 8
STRIDE = 8.0  # 416 / 52
ANCHORS_PX = np.array([[10.0, 13.0], [16.0, 30.0], [33.0, 23.0]], dtype=np.float32)
K_MM = 85  # 2 exp + 2 raw xy + 81 raw conf/cls
R = 22  # output rows per partition
P_OUT = 123  # output partitions per matmul (123*22 = 2706 >= 2704)
FREE = P_OUT * R  # 2706
N_SLABS = B_PER_CORE * A  # 24
NGRP = 24  # psum groups per slab (22 real + 2 dummy)
OUT_COLS = NGRP * NCH  # 2040

_CACHE = {}


def _build_consts():
    mmat = np.zeros((K_MM, NCH), dtype=np.float16)
    mmat[0, 2] = 1.0  # exp(tw)*aw_px -> col 2
    mmat[1, 3] = 1.0  # exp(th)*ah_px -> col 3
    mmat[2, 0] = 1.0  # raw tx -> col 0 (sigmoid'd at drain)
    mmat[3, 1] = 1.0  # raw ty -> col 1
    for k in range(81):  # raw conf + cls -> cols 4..84
        mmat[4 + k, 4 + k] = 1.0

    # 8*cx / 8*cy for grid row 22p + t (multiples of 8: exact in fp16)
    cxpat = np.zeros((P_OUT, NGRP, 2), dtype=np.float16)
    p = np.arange(P_OUT)[:, None]
    t = np.arange(R)[None, :]
    row = R * p + t  # [123, 22]
    valid = row < GG
    cxpat[:, :R, 0] = np.where(valid, STRIDE * (row % G), 0).astype(np.float16)
    cxpat[:, :R, 1] = np.where(valid, STRIDE * (row // G), 0).astype(np.float16)
    return mmat, cxpat


def build_nc():
    if "nc" in _CACHE:
        return _CACHE["nc"]
    from contextlib import ExitStack

    import concourse.bacc as bacc
    import concourse.tile as tile
    from concourse import mybir
    from concourse.tile_rust import add_dep_helper

    AF = mybir.ActivationFunctionType
    dt = mybir.dt

    nc = bacc.Bacc("TRN2", target_bir_lowering=False, debug=False)
    xe_t = nc.dram_tensor(
        "xe", [B_PER_CORE, A, K_MM, FREE], dt.float16, kind="ExternalInput"
    )
    mmat_t = nc.dram_tensor("mmat", [K_MM, NCH], dt.float16, kind="ExternalInput")
    cxpat_t = nc.dram_tensor(
        "cxpat", [P_OUT, NGRP * 2], dt.float16, kind="ExternalInput"
    )
    out_t = nc.dram_tensor(
        "out", [B_PER_CORE, A, FREE, NCH], dt.float16, kind="ExternalOutput"
    )
    xe_ap = xe_t.ap()
    mmat_ap = mmat_t.ap()
    cxpat_ap = cxpat_t.ap()
    out_ap = out_t.ap()

    with ExitStack() as ctx:
        tc = ctx.enter_context(tile.TileContext(nc))
        singles = ctx.enter_context(tc.tile_pool(name="singles", bufs=1))
        slabs = ctx.enter_context(tc.tile_pool(name="slabs", bufs=14))
        outs = ctx.enter_context(tc.tile_pool(name="outs", bufs=5))
        psums = ctx.enter_context(tc.tile_pool(name="psum", bufs=2, space="PSUM"))

        stg_exp = singles.tile([2 * N_SLABS, FREE], dt.float16)
        mmat_sb = singles.tile([K_MM, NCH], dt.float16)
        cxpat_sb = singles.tile([P_OUT, NGRP * 2], dt.float16)

        # SP issues the exp staging input first (it gates the Pool s2s chain
        # and so the whole pipeline fill); everything stays off ACT's
        # sequencer so the exp table load is ACT's first instruction at t~0
        # anchor scale rides in the host pack (tw' = tw + ln(anchor_px)), so
        # the staged exp needs no bias and starts as soon as its rows land
        nc.sync.dma_start(out=stg_exp[:, :], in_=xe_ap[:, :, 0:2, :])
        # const issues ride ACT's sequencer (hidden inside exp's wait for its
        # staging rows) so SP goes straight from staging to the slab loads
        nc.scalar.dma_start(out=mmat_sb[:, :], in_=mmat_ap[:, :])
        nc.scalar.dma_start(out=cxpat_sb[:, :], in_=cxpat_ap[:, :])
        nc.scalar.activation(stg_exp[:, :], stg_exp[:, :], AF.Exp)

        # warm the PE (pipeline + p-state) on the constant matrix while the
        # first slab loads stream in
        wps = psums.tile([P_OUT, 4, 512], dt.float32, tag="ps")
        for _ in range(16):
            nc.tensor.matmul(
                wps[0:NCH, 0, 0:NCH], mmat_sb[:, :], mmat_sb[:, :],
                start=True, stop=True,
            )

        # stores issue from SP two slabs late so their semaphore waits never
        # block the next loads' issue (HWDGE here is SP/ACT only; ACT's
        # sequencer has no slack and Pool's SWDGE engine time is too dear)
        pending_stores = []
        s2s_hist = []

        def flush_store():
            bb, aa, hh, sb_tile = pending_stores.pop(0)
            nc.sync.dma_start(
                out=out_ap[bb, aa, :, :].rearrange("(p r) c -> p (r c)", r=R),
                in_=sb_tile[:, 0 : R * NCH],
            )

        for b in range(B_PER_CORE):
            for a in range(A):
                s = A * b + a
                slab = slabs.tile([K_MM, FREE], dt.float16)
                # exp rows move by DMA on the Pool engine (engine copies need
                # 32-aligned partition bases)
                s2s_i = nc.gpsimd.dma_start(
                    out=slab[0:2, :], in_=stg_exp[2 * s : 2 * s + 2, :]
                )
                s2s_hist.append(s2s_i)
                load_i = nc.sync.dma_start(
                    out=slab[2:K_MM, :], in_=xe_ap[b, a, 2:K_MM, :]
                )
                if s >= 3:
                    # the DMA engines are an exclusive FIFO: without this, the
                    # tiny s2s transfers queue behind every already-issued
                    # slab load during pipeline fill and the first matmuls
                    # (and everything behind them) start ~5us late
                    add_dep_helper(
                        load_i.ins, s2s_hist[s - 3].ins, sync=True,
                        reason="fill: exp-row transfers jump the load queue",
                    )
                if len(pending_stores) >= 6:
                    flush_store()
                # [K_MM, P_OUT, R]: free index (p, t) -> grid row R*p + t
                slab_r = slab[:, :].rearrange("k (p t) -> k p t", t=R)

                ps = psums.tile([P_OUT, 4, 512], dt.float32, tag="ps")
                for k in range(NGRP):
                    # full 123 partitions even for t>=20: pad cols of xe are
                    # zero, so the 2 out-of-range grid rows compute to benign
                    # values (excluded from the store DMA); groups 22/23 are
                    # dummies that keep the psum tile uniformly initialized
                    # for the whole-tile sigmoid drain below
                    t = k if k < R else 0
                    bank, jj = divmod(k, 6)
                    nc.tensor.matmul(
                        ps[:, bank, jj * NCH : (jj + 1) * NCH],
                        slab_r[:, :, t],
                        mmat_sb[:, :],
                        start=True,
                        stop=True,
                    )

                out_sb = outs.tile([P_OUT, OUT_COLS], dt.float16)
                # one whole-tile drain decodes conf/cls AND tx/ty (cols 0:2,
                # 4:85 of every group) while converting f32->fp16; exp cols
                # get sigmoid'd garbage here and are re-overwritten below
                nc.scalar.activation(
                    out_sb[:, :].rearrange("p (b c) -> p b c", c=6 * NCH),
                    ps[:, :, 0 : 6 * NCH],
                    AF.Sigmoid,
                )
                nc.vector.tensor_copy(
                    out_sb[:, :].rearrange("p (b g c) -> p b g c", b=4, c=NCH)[
                        :, :, :, 2:4
                    ],
                    ps[:, :, 0 : 6 * NCH].rearrange("p b (g c) -> p b g c", c=NCH)[
                        :, :, :, 2:4
                    ],
                )
                # bbox xy: 8*sigmoid(t) + 8*grid offset
                xy = out_sb[:, :].rearrange("p (g c) -> p g c", c=NCH)[:, :, 0:2]
                nc.vector.tensor_scalar_mul(xy, xy, 8.0)
                nc.vector.tensor_tensor(
                    xy,
                    xy,
                    cxpat_sb[:, :].rearrange("p (g c) -> p g c", c=2),
                    mybir.AluOpType.add,
                )
                # uniform [123 x 3740B] store; DRAM rows 2704:2706 are pad
                pending_stores.append((b, a, 0, out_sb))
        while pending_stores:
            flush_store()

    nc.compile()
    _CACHE["nc"] = nc
    return nc


def _pack_core_input(x_core):
    """x_core [B_PER_CORE, 255, 52, 52] f32 -> xe [B_PER_CORE, A, 85, FREE] f16."""
    xr = x_core.reshape(B_PER_CORE, A, NCH, GG)
    xe = np.zeros((B_PER_CORE, A, K_MM, FREE), dtype=np.float16)
    law = np.log(ANCHORS_PX[:, 0:1]).astype(np.float32)[None, :, :, None]
    lah = np.log(ANCHORS_PX[:, 1:2]).astype(np.float32)[None, :, :, None]
    xe[:, :, 0:1, 0:GG] = xr[:, :, 2:3] + law  # tw + ln(anchor_w_px)
    xe[:, :, 1:2, 0:GG] = xr[:, :, 3:4] + lah  # th + ln(anchor_h_px)
    xe[:, :, 2:4, 0:GG] = xr[:, :, 0:2]  # tx, ty
    xe[:, :, 4:NCH, 0:GG] = xr[:, :, 4:NCH]  # conf, cls
    return xe


def kernel(x):
    x = np.ascontiguousarray(np.asarray(x), dtype=np.float32)
    assert x.shape == (B, A * NCH, G, G), x.shape
    nc = build_nc()
    from concourse.bass_utils import run_bass_kernel_spmd

    mmat, cxpat = _build_consts()
    in_maps = []
    for c in range(N_CORES):
        in_maps.append(
            {
                "xe": _pack_core_input(x[c * B_PER_CORE : (c + 1) * B_PER_CORE]),
                "mmat": mmat,
                "cxpat": cxpat.reshape(P_OUT, NGRP * 2),
            }
        )
    # transient NRT_EXEC_UNIT_UNRECOVERABLE has been observed once on a cold
    # first execution and never again; retry a couple of times before failing
    for attempt in range(3):
        try:
            res = run_bass_kernel_spmd(nc, in_maps, core_ids=list(range(N_CORES)))
            break
        except Exception:  # noqa: BLE001
            if attempt == 2:
                raise
            import time

            time.sleep(2.0 * (attempt + 1))
    _CACHE["last_res"] = res
    out = np.concatenate([r["out"] for r in res.results], axis=0)
    out = out[:, :, 0:GG, :].astype(np.float32)
    return out.reshape(B, A * GG, NCH)


# revision 23
# speedup vs baseline: 1.1038x; 1.0061x over previous
"""YOLO-style detection head decode on 8 Trainium2 NeuronCores.

Input : x [64, 255, 52, 52] f32
Output: [64, 8112, 85] f32  (bbox(4) | conf(1) | cls(80), sigmoid/exp decoded)

The kernel is DMA-bound (360 B/ns shared DMA engines), so everything rides
the wire as fp16 (rel err ~2e-3 vs the 2e-2 gate; bf16's 7-bit mantissa
fails at sigmoid tails).  Per core: 8 batches x 3 anchors = 24 slabs.

  - host packs xe [8, 3, 85, 2706] fp16 per slab (grid padded 2704->2706):
      rows 0/1  = raw tw, th          (decoded by the staged exp below)
      rows 2/3  = raw tx, ty          (sigmoid applied POST-transpose)
      rows 4..84 = raw conf, cls0..79 (sigmoid applied POST-transpose)
  - tw/th exp(+ln anchor bias) runs as ONE [48, 2706] ACT op on a staging
    tile (a per-slab [2, 2706] op would cost the same 2.3us each on ACT's
    free-dim clock), then moves into slab rows 0:2 by SBUF->SBUF DMA on the
    otherwise-idle Pool engine (24 transfers; keep this count low — each
    SWDGE op costs ~1us of Pool descriptor-generation time and the fill
    phase advances at that pace).
  - per slab: one 83-row HBM load (rows 2:85), 22 fp16 matmuls (+2 dummies
    to fill the 24-group psum) transpose to output layout: lhsT free dim
    strided by 22 so each of 123 output partitions holds 22 consecutive
    output rows; rhs = constant [85, 85] permutation (exp rows -> cols 2/3,
    tx/ty -> cols 0/1, raw rows -> cols 4..84).
  - psum [123, 4, 512] f32 (4 banks, 6 x 85-col groups per bank) drains via
    ONE whole-tile ACT sigmoid [123, (4, 510)] f32->fp16 — decoding conf,
    cls AND tx/ty while transposing — then per bank a small DVE copy
    re-overwrites exp cols 2:4 of each group with raw psum values, and two
    DVE ops finish bbox xy: out[:, t, 0:2] = 8*sigmoid + 8*grid (pattern
    add from a tiny constant tile).
  - out DRAM padded to [.., 2706, 85] so each slab stores with a single
    uniform [123 x 3740B] DMA issued from SP two slabs late (so its
    semaphore wait never blocks the next loads' issue); host drops the 2
    pad rows when gathering.
"""

import numpy as np

G = 52
GG = G * G  # 2704
A = 3
NCH = 85  # 5 + 80
B = 64
N_CORES = 8
B_PER_CORE = B // N_CORES  # 8
STRIDE = 8.0  # 416 / 52
ANCHORS_PX = np.array([[10.0, 13.0], [16.0, 30.0], [33.0, 23.0]], dtype=np.float32)
K_MM = 85  # 2 exp + 2 raw xy + 81 raw conf/cls
R = 22  # output rows per partition
P_OUT = 123  # output partitions per matmul (123*22 = 2706 >= 2704)
FREE = P_OUT * R  # 2706
N_SLABS = B_PER_CORE * A  # 24
NGRP = 24  # psum groups per slab (22 real + 2 dummy)
OUT_COLS = NGRP * NCH  # 2040

_CACHE = {}


def _build_consts():
    mmat = np.zeros((K_MM, NCH), dtype=np.float16)
    mmat[0, 2] = 1.0  # exp(tw)*aw_px -> col 2
    mmat[1, 3] = 1.0  # exp(th)*ah_px -> col 3
    mmat[2, 0] = 1.0  # raw tx -> col 0 (sigmoid'd at drain)
    mmat[3, 1] = 1.0  # raw ty -> col 1
    for k in range(81):  # raw conf + cls -> cols 4..84
        mmat[4 + k, 4 + k] = 1.0

    # 8*cx / 8*cy for grid row 22p + t (multiples of 8: exact in fp16)
    cxpat = np.zeros((P_OUT, NGRP, 2), dtype=np.float16)
    p = np.arange(P_OUT)[:, None]
    t = np.arange(R)[None, :]
    row = R * p + t  # [123, 22]
    valid = row < GG
    cxpat[:, :R, 0] = np.where(valid, STRIDE * (row % G), 0).astype(np.float16)
    cxpat[:, :R, 1] = np.where(valid, STRIDE * (row // G), 0).astype(np.float16)
    return mmat, cxpat


def build_nc():
    if "nc" in _CACHE:
        return _CACHE["nc"]
    from contextlib import ExitStack

    import concourse.bacc as bacc
    import concourse.tile as tile
    from concourse import mybir
    from concourse.tile_rust import add_dep_helper

    AF = mybir.ActivationFunctionType
    dt = mybir.dt

    nc = bacc.Bacc("TRN2", target_bir_lowering=False, debug=False)
    xe_t = nc.dram_tensor(
        "xe", [B_PER_CORE, A, K_MM, FREE], dt.float16, kind="ExternalInput"
    )
    mmat_t = nc.dram_tensor("mmat", [K_MM, NCH], dt.float16, kind="ExternalInput")
    cxpat_t = nc.dram_tensor(
        "cxpat", [P_OUT, NGRP * 2], dt.float16, kind="ExternalInput"
    )
    out_t = nc.dram_tensor(
        "out", [B_PER_CORE, A, FREE, NCH], dt.float16, kind="ExternalOutput"
    )
    xe_ap = xe_t.ap()
    mmat_ap = mmat_t.ap()
    cxpat_ap = cxpat_t.ap()
    out_ap = out_t.ap()

    with ExitStack() as ctx:
        tc = ctx.enter_context(tile.TileContext(nc))
        singles = ctx.enter_context(tc.tile_pool(name="singles", bufs=1))
        slabs = ctx.enter_context(tc.tile_pool(name="slabs", bufs=14))
        outs = ctx.enter_context(tc.tile_pool(name="outs", bufs=6))
        psums = ctx.enter_context(tc.tile_pool(name="psum", bufs=2, space="PSUM"))

        stg_exp = singles.tile([2 * N_SLABS, FREE], dt.float16)
        mmat_sb = singles.tile([K_MM, NCH], dt.float16)
        cxpat_sb = singles.tile([P_OUT, NGRP * 2], dt.float16)

        # SP issues the exp staging input first (it gates the Pool s2s chain
        # and so the whole pipeline fill); everything stays off ACT's
        # sequencer so the exp table load is ACT's first instruction at t~0
        # anchor scale rides in the host pack (tw' = tw + ln(anchor_px)), so
        # the staged exp needs no bias and starts as soon as its rows land
        nc.sync.dma_start(out=stg_exp[:, :], in_=xe_ap[:, :, 0:2, :])
        # const issues ride ACT's sequencer (hidden inside exp's wait for its
        # staging rows) so SP goes straight from staging to the slab loads
        nc.scalar.dma_start(out=mmat_sb[:, :], in_=mmat_ap[:, :])
        nc.scalar.dma_start(out=cxpat_sb[:, :], in_=cxpat_ap[:, :])
        nc.scalar.activation(stg_exp[:, :], stg_exp[:, :], AF.Exp)

        # warm the PE (pipeline + p-state) on the constant matrix while the
        # first slab loads stream in
        wps = psums.tile([P_OUT, 4, 512], dt.float32, tag="ps")
        for _ in range(16):
            nc.tensor.matmul(
                wps[0:NCH, 0, 0:NCH], mmat_sb[:, :], mmat_sb[:, :],
                start=True, stop=True,
            )

        # stores issue from SP two slabs late so their semaphore waits never
        # block the next loads' issue (HWDGE here is SP/ACT only; ACT's
        # sequencer has no slack and Pool's SWDGE engine time is too dear)
        pending_stores = []
        s2s_hist = []

        def flush_store():
            bb, aa, hh, sb_tile = pending_stores.pop(0)
            nc.sync.dma_start(
                out=out_ap[bb, aa, :, :].rearrange("(p r) c -> p (r c)", r=R),
                in_=sb_tile[:, 0 : R * NCH],
            )

        for b in range(B_PER_CORE):
            for a in range(A):
                s = A * b + a
                slab = slabs.tile([K_MM, FREE], dt.float16)
                # exp rows move by DMA on the Pool engine (engine copies need
                # 32-aligned partition bases)
                s2s_i = nc.gpsimd.dma_start(
                    out=slab[0:2, :], in_=stg_exp[2 * s : 2 * s + 2, :]
                )
                s2s_hist.append(s2s_i)
                load_i = nc.sync.dma_start(
                    out=slab[2:K_MM, :], in_=xe_ap[b, a, 2:K_MM, :]
                )
                if s >= 3:
                    # the DMA engines are an exclusive FIFO: without this, the
                    # tiny s2s transfers queue behind every already-issued
                    # slab load during pipeline fill and the first matmuls
                    # (and everything behind them) start ~5us late
                    add_dep_helper(
                        load_i.ins, s2s_hist[s - 3].ins, sync=True,
                        reason="fill: exp-row transfers jump the load queue",
                    )
                if len(pending_stores) >= 5:
                    flush_store()
                # [K_MM, P_OUT, R]: free index (p, t) -> grid row R*p + t
                slab_r = slab[:, :].rearrange("k (p t) -> k p t", t=R)

                ps = psums.tile([P_OUT, 4, 512], dt.float32, tag="ps")
                for k in range(NGRP):
                    # full 123 partitions even for t>=20: pad cols of xe are
                    # zero, so the 2 out-of-range grid rows compute to benign
                    # values (excluded from the store DMA); groups 22/23 are
                    # dummies that keep the psum tile uniformly initialized
                    # for the whole-tile sigmoid drain below
                    t = k if k < R else 0
                    bank, jj = divmod(k, 6)
                    nc.tensor.matmul(
                        ps[:, bank, jj * NCH : (jj + 1) * NCH],
                        slab_r[:, :, t],
                        mmat_sb[:, :],
                        start=True,
                        stop=True,
                    )

                out_sb = outs.tile([P_OUT, OUT_COLS], dt.float16)
                # one whole-tile drain decodes conf/cls AND tx/ty (cols 0:2,
                # 4:85 of every group) while converting f32->fp16; exp cols
                # get sigmoid'd garbage here and are re-overwritten below
                nc.scalar.activation(
                    out_sb[:, :].rearrange("p (b c) -> p b c", c=6 * NCH),
                    ps[:, :, 0 : 6 * NCH],
                    AF.Sigmoid,
                )
                nc.vector.tensor_copy(
                    out_sb[:, :].rearrange("p (b g c) -> p b g c", b=4, c=NCH)[
                        :, :, :, 2:4
                    ],
                    ps[:, :, 0 : 6 * NCH].rearrange("p b (g c) -> p b g c", c=NCH)[
                        :, :, :, 2:4
                    ],
                )
                # bbox xy: 8*sigmoid(t) + 8*grid offset
                xy = out_sb[:, :].rearrange("p (g c) -> p g c", c=NCH)[:, :, 0:2]
                nc.vector.tensor_scalar_mul(xy, xy, 8.0)
                nc.vector.tensor_tensor(
                    xy,
                    xy,
                    cxpat_sb[:, :].rearrange("p (g c) -> p g c", c=2),
                    mybir.AluOpType.add,
                )
                # uniform [123 x 3740B] store; DRAM rows 2704:2706 are pad
                pending_stores.append((b, a, 0, out_sb))
        while pending_stores:
            flush_store()

    nc.compile()
    _CACHE["nc"] = nc
    return nc


def _pack_core_input(x_core):
    """x_core [B_PER_CORE, 255, 52, 52] f32 -> xe [B_PER_CORE, A, 85, FREE] f16."""
    xr = x_core.reshape(B_PER_CORE, A, NCH, GG)
    xe = np.zeros((B_PER_CORE, A, K_MM, FREE), dtype=np.float16)
    law = np.log(ANCHORS_PX[:, 0:1]).astype(np.float32)[None, :, :, None]
    lah = np.log(ANCHORS_PX[:, 1:2]).astype(np.float32)[None, :, :, None]
    xe[:, :, 0:1, 0:GG] = xr[:, :, 2:3] + law  # tw + ln(anchor_w_px)
    xe[:, :, 1:2, 0:GG] = xr[:, :, 3:4] + lah  # th + ln(anchor_h_px)
    xe[:, :, 2:4, 0:GG] = xr[:, :, 0:2]  # tx, ty
    xe[:, :, 4:NCH, 0:GG] = xr[:, :, 4:NCH]  # conf, cls
    return xe


def kernel(x):
    x = np.ascontiguousarray(np.asarray(x), dtype=np.float32)
    assert x.shape == (B, A * NCH, G, G), x.shape
    nc = build_nc()
    from concourse.bass_utils import run_bass_kernel_spmd

    mmat, cxpat = _build_consts()
    in_maps = []
    for c in range(N_CORES):
        in_maps.append(
            {
                "xe": _pack_core_input(x[c * B_PER_CORE : (c + 1) * B_PER_CORE]),
                "mmat": mmat,
                "cxpat": cxpat.reshape(P_OUT, NGRP * 2),
            }
        )
    # transient NRT_EXEC_UNIT_UNRECOVERABLE has been observed once on a cold
    # first execution and never again; retry a couple of times before failing
    for attempt in range(3):
        try:
            res = run_bass_kernel_spmd(nc, in_maps, core_ids=list(range(N_CORES)))
            break
        except Exception:  # noqa: BLE001
            if attempt == 2:
                raise
            import time

            time.sleep(2.0 * (attempt + 1))
    _CACHE["last_res"] = res
    out = np.concatenate([r["out"] for r in res.results], axis=0)
    out = out[:, :, 0:GG, :].astype(np.float32)
    return out.reshape(B, A * GG, NCH)
